# revision 87
# baseline (speedup 1.0000x reference)
# Trainium2 Bass kernel for a causal multi-head attention block.
#
# Reference computation (fp32):
#   qkv = x @ w_attn + b_attn ; split into q,k,v heads (N=16, H=64)
#   scores = q @ k^T / sqrt(H), causal mask, softmax over keys
#   out = (weights @ v) reshaped, then out @ w_proj + b_proj
#
# Sharding: 8 cores = 2 batches x 4 head-groups (4 heads each).
#   - batch data-parallel, heads tensor-parallel (c_attn columns / c_proj rows)
#   - each core emits a partial [T, D] projection output (bf16); host sums the
#     4 head-group partials per batch in f32 and adds b_proj (the gather step).
#
# v4 design notes (fp8 DoubleRow datapath):
#   - The TimelineSim cost model charges matmuls ap_size(out-free) x
#     cycles_per_row; fp8e4 with perf_mode=DoubleRow runs at 0.5 cyc/row and
#     contracts 2 k-tiles per matmul (effective 256-deep contraction), i.e.
#     4x the bf16 MAC rate on 128-deep chains.
#   - qk generation uses an error-compensated fp8 split:
#       q = x8@w8 + xr8@w8 + x8@wr8   (xr8/wr8 = fp8 residuals of x/w)
#     3 fp8 terms cost 0.75x one bf16 chain but carry ~bf16 accuracy.
#   - v generation compensates the w-side residual (its quant noise is
#     correlated across keys and does NOT average out under the softmax);
#     the first four key-tiles also compensate the x-side residual because
#     early rows average too few keys to wash it out.
#   - scores run fp8 DoubleRow with a 32-partition layout: qk psum drains
#     straight to fp8 tiles [128,1024] and two tiny SBUF->SBUF DMAs shift
#     head-half partitions 32:64 -> 0:32 (cols 512:1024), giving lhsT/rhs
#     APs [32p, 2kt, cols]. t-block 0 keeps a bf16 scores path so the
#     relayout DMA latency never sits on the kernel-startup critical path.
#   - softmax exp stays on ACT (the bottleneck engine, ~73us busy): the
#     diagonal-mask multiplies move to GPSIMD and the per-block softmax
#     normalize is batched into 2 DVE ops per (tb, hp) so DVE stays clear.
#   - w columns and stored q/k are pre-scaled by WS=32 on the host to sit in
#     fp8e4's normal range (max finite 240); the exp scale folds 1/WS^2.

import math

import numpy as np

B, T, D = 2, 2048, 1024
NHEAD, H = 16, 64
HPC = 4            # heads per core
CD = HPC * H       # 256 head-dim columns per core
N_CORES = 8
P = 128            # partitions
TB = T // 512      # 4 t-blocks of 512
KD = D // P        # 8 contraction tiles over D
G = H + 1          # AV output cols per head (64 + ones-col for rowsum)
WS = 32.0          # host-side fp8 scale on w_attn cols and stored q/k

_CACHE = {}


def _build_module():
    import contextlib

    import concourse.bass as bass  # noqa: F401
    import concourse.mybir as mybir
    import concourse.tile as tile
    from concourse import bacc

    f32 = mybir.dt.float32
    bf = mybir.dt.bfloat16
    f8 = mybir.dt.float8e4
    DR = mybir.MatmulPerfMode.DoubleRow

    nc = bacc.Bacc("TRN2", target_bir_lowering=False, debug=False)

    # xall packs [x8 | xr8] columns; wall packs the six fp8 weight groups
    # [w8-q0k0 | wr8-q0k0 | w8-v | wr8-v | w8-q1k1 | wr8-q1k1] so the whole
    # load plan is a handful of big DMAs (the DMA front-end is config-bound).
    xall_d = nc.dram_tensor("xall", [D, 2 * T], f8, kind="ExternalInput").ap()
    wall_d = nc.dram_tensor("wall", [D, 12 * P], f8, kind="ExternalInput").ap()
    # consts packed per-partition into one tensor/DMA (the DMA front-end
    # charges ~1.4us per instruction): f32 view cols = [bqk 4 | bv 256],
    # then bf16 cols [ident 128 | mask 128 | ones 4]
    CB = (4 + CD) * 4 + (P + P + HPC) * 2  # bytes per partition
    consts_d = nc.dram_tensor("consts", [P, CB], mybir.dt.uint8,
                              kind="ExternalInput").ap()
    wp_d = nc.dram_tensor("wp", [CD, D], bf, kind="ExternalInput").ap()
    y_d = nc.dram_tensor("y", [T, D], bf, kind="ExternalOutput").ap()

    with tile.TileContext(nc) as tc, contextlib.ExitStack() as ctx:
        const_p = ctx.enter_context(tc.tile_pool(name="const", bufs=1))
        w_p = ctx.enter_context(tc.tile_pool(name="weights", bufs=1))
        xt_p = ctx.enter_context(tc.tile_pool(name="xt", bufs=1))
        qkt_p = ctx.enter_context(tc.tile_pool(name="qkt", bufs=1))
        v_p = ctx.enter_context(tc.tile_pool(name="vbuf", bufs=1))
        e_p = ctx.enter_context(tc.tile_pool(name="epool", bufs=24))
        attn_p = ctx.enter_context(tc.tile_pool(name="attn", bufs=1))
        at_p = ctx.enter_context(tc.tile_pool(name="attnT", bufs=1))
        ysb_p = ctx.enter_context(tc.tile_pool(name="ysb", bufs=8))
        small_p = ctx.enter_context(tc.tile_pool(name="small", bufs=8))
        # single PSUM pool, 8 banks total:
        #   wps  [128,512]f32 x2  (qk/v gen accum + proj jb<3)       2 banks
        #   sp   [128,1024]f32 x2 (scores, 2 heads side by side)     4 banks
        #   accp [128,260]f32 x2  (AV accumulators, 4 groups each)   2 banks
        #   tp   [128,512]bf16 x1 (attn^T transposes)                in slack
        psp = ctx.enter_context(tc.tile_pool(name="psp", bufs=2, space="PSUM"))

        # ---- PE warmup ----
        # ~3us of junk matmuls on a memset tile during the initial DMA
        # latency window so the p-state ramp is paid on junk work. A dummy
        # activation pulls the ACT exp-table load into the same window.
        junk = const_p.tile([P, 5 * P], bf, name="junk_sb")
        nc.gpsimd.memset(junk, 0)
        dummy = const_p.tile([P, 1], bf, name="dummy_sb")
        nc.scalar.activation(dummy, junk[:, 0:1],
                             mybir.ActivationFunctionType.Exp)
        jps = psp.tile([P, 1024], f32, name="warm", tag="sp")
        for _ in range(8):
            nc.tensor.matmul(jps[:, 0:512], junk[:, 0:P], junk[:, P:],
                             start=True, stop=True)

        # ---- loads ----
        wall_sb = w_p.tile([P, KD * 12 * P], f8, name="wall_sb")
        wall_v = wall_sb.rearrange("p (k c) -> p k c", k=KD)
        xall_sb = xt_p.tile([P, KD * 2 * T], f8, name="xall_sb")
        xall_v = xall_sb.rearrange("p (k t) -> p k t", k=KD)
        # wall column map (host order): [w8 q0|k0, wr8 q0|k0, w8 v, wr8 v,
        # w8 q1|k1, wr8 q1|k1]; xall columns interleave per 512-t-block:
        # [j0: x8|xr8, j1: x8|xr8, ...] so each j-load is contiguous.
        W8COL = {0: 0, 2: P, 1: 8 * P, 3: 9 * P}
        WRCOL = {0: 2 * P, 2: 3 * P, 1: 10 * P, 3: 11 * P}
        VCOL = 4 * P
        VRCOL = 6 * P

        def XCOL(j, r):
            return 1024 * j + 512 * r

        def load_w(c0, c1):
            nc.sync.dma_start(
                wall_v[:, :, c0:c1],
                wall_d[:, c0:c1].rearrange("(k p) c -> p k c", p=P),
            )

        def load_xj(j0, j1):
            # x8 j-blocks + matching xr8 j-blocks in one contiguous DMA
            nc.sync.dma_start(
                xall_v[:, :, 1024 * j0 : 1024 * j1],
                xall_d[:, 1024 * j0 : 1024 * j1].rearrange(
                    "(k p) t -> p k t", p=P
                ),
            )

        # Everything rides the sync (SP) ring: the DMA bus is one serial
        # resource in the cost model, ring-splitting buys nothing, and any
        # DMA on the scalar ring parks on the ACT sequencer (a config that
        # waits for a ring slot blocks exp decode for MILLIseconds of sim
        # time). Loads are few and big, in strict consumer order; the two
        # bulk tails (x-j2|j3, wp) are emitted AFTER the first qk chains so
        # the dynamically emitted qkt8 relayout DMAs aren't queued behind
        # their 6us of transfer.
        load_w(0, 4 * P)     # w8 + wr8 q0|k0 (first chains' weights)
        nc.sync.dma_start(   # x8 j0 alone: the j0 2-term chains need just it
            xall_v[:, :, 0:512],
            xall_d[:, 0:512].rearrange("(k p) t -> p k t", p=P),
        )
        cons = const_p.tile([P, CB], mybir.dt.uint8, name="consts_sb")
        nc.sync.dma_start(cons, consts_d)
        consf = cons.bitcast(f32)
        bqk = consf[:, 0:4]
        bv = consf[:, 4 : 4 + CD]
        consb = cons.bitcast(bf)
        CB2 = (4 + CD) * 2  # bf16-element offset of the bf16 section
        ident = consb[:, CB2 : CB2 + P]
        mask = consb[:, CB2 + P : CB2 + 2 * P]
        onescol = consb[:, CB2 + 2 * P : CB2 + 2 * P + HPC]
        nc.sync.dma_start(   # xr8 j0 (term 3 of the startup chains)
            xall_v[:, :, 512:1024],
            xall_d[:, 512:1024].rearrange("(k p) t -> p k t", p=P),
        )
        load_w(4 * P, 12 * P)  # w8+wr8 v, w8+wr8 q1|k1
        load_xj(1, 2)
        wp_t = w_p.tile([P, 2 * D], bf, name="wp_sb")
        wp_sb = [wp_t[:, 0:D], wp_t[:, D : 2 * D]]

        def load_tail():
            load_xj(2, 3)
            load_xj(3, 4)
            nc.sync.dma_start(
                wp_t.rearrange("p (c d) -> p c d", c=2),
                wp_d.rearrange("(c p) d -> p c d", p=P),
            )

        # persistent activation buffers
        # qkt8[(m, j)]: fp8, [128, 1024]; cols 0:512 = drained 32*q (or 32*k),
        # cols 512:1024 = partition-shifted copy (head-half 1) for DoubleRow.
        qkt8 = {}
        for m in range(4):
            for j in range(TB):
                qkt8[(m, j)] = qkt_p.tile(
                    [P, 1024], f8, name=f"qkt8_{m}_{j}", tag=f"qkt8_{m}_{j}"
                )
        # bf16 qk tiles for j=0 (t-block 0 scores skip the fp8 relayout)
        qkt0 = {}
        for m in range(4):
            qkt0[m] = qkt_p.tile([P, 512], bf, name=f"qkt0_{m}", tag=f"qkt0_{m}")
        v_sb = []
        for i in range(T // P):
            v_sb.append(v_p.tile([P, HPC * G], bf, name=f"v{i}", tag=f"v{i}"))
        attn_t = {
            (tb, hp): attn_p.tile([P, 512], bf, name=f"attn{tb}_{hp}",
                                  tag=f"attn{tb}_{hp}")
            for tb in range(TB)
            for hp in range(2)
        }
        attnT = {
            (tb, hp): at_p.tile([P, 512], bf, name=f"at{tb}_{hp}",
                                tag=f"at{tb}_{hp}")
            for tb in range(TB)
            for hp in range(2)
        }

        # ones-columns in V are static: set them once upfront on gpsimd
        for i in range(T // P):
            nc.gpsimd.tensor_copy(
                v_sb[i].rearrange("p (g c) -> p g c", g=HPC)[:, :, H:G],
                onescol.rearrange("p (g c) -> p g c", c=1),
            )

        def qk_group(m, j, chunk=99):
            """Generator: compensated-fp8 qk chain for (m, j); yields every
            `chunk` matmuls so the caller can interleave score matmuls.
            Drains to fp8 (j>0) with the partition-shift relayout DMAs, and
            to bf16 for j==0 (m 0,1) or both (m 2,3: k tiles feed all tb)."""
            ps = psp.tile([P, 512], f32, name="qkp", tag="wps")
            n = 0
            for wc, xr in ((W8COL[m], 0), (WRCOL[m], 0), (W8COL[m], 1)):
                for kk in range(4):
                    nc.tensor.matmul(
                        ps,
                        wall_v[:, 2 * kk : 2 * kk + 2, wc : wc + P],
                        xall_v[:, 2 * kk : 2 * kk + 2,
                               XCOL(j, xr) : XCOL(j, xr) + 512],
                        start=(n == 0),
                        stop=(n == 11),
                        perf_mode=DR,
                    )
                    n += 1
                    if n % chunk == 0 and n < 12:
                        yield
            # psum->sbuf drain(s): j=0 q-tiles go to bf16 (t-block 0 scores
            # skip the fp8 relayout); j=0 k-tiles dual-drain (bf16 for tb0
            # + fp8/relayout for tb>=1); j>0 drains straight to fp8.
            if j == 0:
                nc.vector.tensor_scalar_add(qkt0[m], ps, bqk[:, m : m + 1])
            if j > 0 or m >= 2:
                t8 = qkt8[(m, j)]
                nc.vector.tensor_scalar_add(t8[:, 0:512], ps, bqk[:, m : m + 1])
                # partition shift for DoubleRow scores: head-half 1 -> cols
                # 512:1024 of head-half 0's partitions. SWDGE (gpsimd):
                # these are latency-critical mid-stream and must not queue
                # behind bulk-load transfers on the sync ring.
                nc.gpsimd.dma_start(t8[0:32, 512:1024], t8[32:64, 0:512])
                nc.gpsimd.dma_start(t8[64:96, 512:1024], t8[96:128, 0:512])

        def v_group(g, chunk=99):
            # fp8 terms: x8@w8v + x8@wr8v (the w-quant noise is correlated
            # across keys -- same dw for every s -- and would NOT average
            # out under the softmax weighting, so it is always compensated).
            # The x-side noise is per-key and washes out over many keys,
            # EXCEPT for the early rows (t < 512) whose softmax spans few
            # keys: the first four key-tiles also get the xr8@w8v term.
            j, ti = g // 4, g % 4
            ps = psp.tile([P, 512], f32, name="vp", tag="wps")
            terms = [(0, VCOL), (0, VRCOL)]
            if g < 4:
                terms.append((1, VCOL))
            nt = 4 * len(terms)
            n = 0
            for xr, vc in terms:
                for kk in range(4):
                    nc.tensor.matmul(
                        ps[:, 0:CD],
                        xall_v[:, 2 * kk : 2 * kk + 2,
                               XCOL(j, xr) + P * ti : XCOL(j, xr) + P * (ti + 1)],
                        wall_v[:, 2 * kk : 2 * kk + 2, vc : vc + CD],
                        start=(n == 0),
                        stop=(n == nt - 1),
                        perf_mode=DR,
                    )
                    n += 1
                    if n % chunk == 0 and n < nt:
                        yield
            # psum->sbuf drain: descale by 1/WS and add the broadcast V bias
            vg = v_sb[g].rearrange("p (g c) -> p g c", g=HPC)
            nc.vector.scalar_tensor_tensor(
                vg[:, :, 0:H],
                ps[:, 0:CD].rearrange("p (g c) -> p g c", g=HPC),
                1.0 / WS,
                bv.rearrange("p (g c) -> p g c", g=HPC),
                mybir.AluOpType.mult,
                mybir.AluOpType.add,
            )

        def run_all(gen):
            for _ in gen:
                pass

        ESC = 1.0 / (math.sqrt(H) * WS * WS)  # exp scale incl. 1/WS^2

        def attention_hp(tb, hp, step_cb=None):
            """S^T -> exp -> AV for one head pair of 512-wide t-block tb;
            one psum tile holds both heads' scores so a single exp covers
            both. Scores are fp8 DoubleRow (bf16 for tb==0).

            Generator protocol (driven by `start_block` below): emits
            score(0), yields; then runs the i-loop with score matmuls
            software-pipelined one step ahead of the exp stream, yields;
            then emits the normalize/transpose tail. The driver slots the
            NEXT block's first score matmuls into the gaps so the ACT exp
            stream never waits on the in-order PE queue."""
            mq, mk = hp, 2 + hp
            n_s = 4 * tb + 4  # s-tiles 0 .. 4*tb+3
            sps_t = {}

            def emit_score(i):
                first = max(0, i - 4 * tb)
                c0 = P * first
                sps = psp.tile([P, 1024], f32, name="sp", tag="sp", bufs=2)
                for hh, pb in ((0, 0), (1, 64)):
                    if tb == 0:
                        nc.tensor.matmul(
                            sps[:, 512 * hh + c0 : 512 * hh + 512],
                            qkt0[mk][pb : pb + H, P * (i % 4) : P * (i % 4 + 1)],
                            qkt0[mq][pb : pb + H, c0:512],
                            start=True,
                            stop=True,
                        )
                    else:
                        kv = qkt8[(mk, i // 4)].rearrange("p (k c) -> p k c", k=2)
                        qv = qkt8[(mq, tb)].rearrange("p (k c) -> p k c", k=2)
                        hb = 64 * hh
                        nc.tensor.matmul(
                            sps[:, 512 * hh + c0 : 512 * hh + 512],
                            kv[hb : hb + 32, :, P * (i % 4) : P * (i % 4 + 1)],
                            qv[hb : hb + 32, :, c0:512],
                            start=True,
                            stop=True,
                            perf_mode=DR,
                        )
                sps_t[i] = sps

            emit_score(0)
            yield  # driver emits the previous block's tail here
            acc_t = [
                psp.tile([P, 4 * G], f32, name="accp", tag=f"accp{a}", bufs=1)
                for a in range(2)
            ]
            for i in range(n_s):
                first = max(0, i - 4 * tb)  # first valid jj in block
                c0 = P * first
                if i + 1 < n_s:
                    emit_score(i + 1)
                sps = sps_t.pop(i)
                if step_cb is not None:
                    step_cb(i)
                et = e_p.tile([P, 1024], bf, name="et", tag="et")
                if first:
                    nc.scalar.activation(
                        et.rearrange("p (g c) -> p g c", g=2)[:, :, c0:512],
                        sps.rearrange("p (g c) -> p g c", g=2)[:, :, c0:512],
                        mybir.ActivationFunctionType.Exp,
                        scale=ESC,
                    )
                else:
                    nc.scalar.activation(
                        et, sps, mybir.ActivationFunctionType.Exp, scale=ESC
                    )
                dj = i - 4 * tb  # diagonal jj of this s-tile, if any
                etd = None
                if 0 <= dj <= 3:
                    # masked diagonal sub-tiles (both heads in one DVE op;
                    # Pool is reserved for the relayout SWDGE gens)
                    etd = e_p.tile([P, 2 * P], bf, name="etd", tag="etd",
                                   bufs=4)
                    nc.vector.tensor_tensor(
                        etd.rearrange("p (g c) -> p g c", g=2),
                        et.rearrange("p (g c) -> p g c", g=2)[
                            :, :, P * dj : P * (dj + 1)
                        ],
                        mask.rearrange("p (o c) -> p o c", o=1)
                        .broadcast_to([P, 2, P]),
                        mybir.AluOpType.mult,
                    )
                for jj in range(first, 4):
                    jglob = 4 * tb + jj
                    for hh in range(2):
                        if jj == dj:
                            lhs_e = etd[:, P * hh : P * (hh + 1)]
                        else:
                            lhs_e = et[
                                :, 512 * hh + P * jj : 512 * hh + P * (jj + 1)
                            ]
                        # start=True clears has_written for the WHOLE psum
                        # bank: only the first group per bank issues it.
                        nc.tensor.matmul(
                            acc_t[hh][:, G * jj : G * jj + G],
                            lhs_e,
                            v_sb[i][:, G * (2 * hp + hh) : G * (2 * hp + hh) + G],
                            start=(i == 0 and jj == 0),
                            stop=(i == jglob),
                            skip_group_check=True,
                        )
            yield  # driver emits the next block's first scores here
            # batched softmax normalize: one reciprocal + one tensor_tensor
            # per head (stride-0 broadcast of the per-group reciprocal)
            for hh in range(2):
                accv = acc_t[hh].rearrange("p (j g) -> p j g", g=G)
                rec4 = small_p.tile([P, 4], f32, name="rec4", tag="rec4")
                nc.vector.reciprocal(
                    rec4.rearrange("p (j g) -> p j g", g=1),
                    accv[:, :, H : H + 1],
                )
                nc.vector.tensor_tensor(
                    attn_t[(tb, hp)]
                    .rearrange("p (j c) -> p j c", c=P)[:, :, H * hh : H * (hh + 1)],
                    accv[:, :, 0:H],
                    rec4.rearrange("p (j g) -> p j g", g=1).broadcast_to(
                        [P, 4, H]
                    ),
                    mybir.AluOpType.mult,
                )
            # attn^T for the projection: bf16 PE transposes reusing this head
            # pair's just-drained AV accumulator bank (bitcast f32->bf16 view)
            pt = psp.tile([P, 4 * G], f32, name="atp", tag="accp0",
                          bufs=1).bitcast(bf)
            for dj in range(4):
                nc.tensor.transpose(
                    pt[:, P * dj : P * (dj + 1)],
                    attn_t[(tb, hp)][:, P * dj : P * (dj + 1)],
                    ident,
                )
            nc.vector.tensor_copy(attnT[(tb, hp)], pt[:, 0:512])

        def proj_gen(jb):
            """Mid-stream projection for t-block jb (pumped while later
            blocks' attention streams): wps psum, DVE drains, sync stores.
            Only t-block 3 keeps the tail-optimized projection() path."""
            for jl in range(4):
                jt = 4 * jb + jl
                ysb = ysb_p.tile([P, D], bf, name="ysb", tag="ysb")
                for n in range(2):
                    ps = psp.tile([P, 512], f32, name="yp", tag="wps")
                    for hp in range(2):
                        nc.tensor.matmul(
                            ps,
                            attnT[(jb, hp)][:, P * jl : P * (jl + 1)],
                            wp_sb[hp][:, 512 * n : 512 * (n + 1)],
                            start=(hp == 0),
                            stop=(hp == 1),
                        )
                    nc.vector.tensor_copy(ysb[:, 512 * n : 512 * (n + 1)], ps)
                nc.sync.dma_start(y_d[P * jt : P * (jt + 1), :], ysb)
                if jl < 3:
                    yield

        def projection(jb):
            """y = attn @ wp for 512-wide t-block jb."""
            for jl in range(4):
                jt = 4 * jb + jl
                ysb = ysb_p.tile([P, D], bf, name="ysb", tag="ysb")
                # proj(3) spreads over BOTH psum rings (all free by then) so
                # four chains are in flight and the tail isn't ring-paced
                merged = (jb == 2) or (jb == 3 and jl == 2)
                if merged:
                    pss = psp.tile([P, 1024], f32, name="yp", tag="sp")
                for n in range(2):
                    if merged:
                        ps = pss[:, 512 * n : 512 * (n + 1)]
                    else:
                        ps = psp.tile([P, 512], f32, name="yp", tag="wps")
                    for hp in range(2):
                        nc.tensor.matmul(
                            ps,
                            attnT[(jb, hp)][:, P * jl : P * (jl + 1)],
                            wp_sb[hp][:, 512 * n : 512 * (n + 1)],
                            start=(hp == 0),
                            stop=(hp == 1),
                        )
                    if jb == 3 and jl == 3:
                        # last tile: half-drain + half-store per engine/ring
                        if n == 0:
                            nc.vector.tensor_copy(ysb[:, 0:512], ps)
                            nc.sync.dma_start(
                                y_d[P * jt : P * (jt + 1), 0:512], ysb[:, 0:512]
                            )
                        else:
                            nc.scalar.copy(ysb[:, 512:1024], ps)
                            nc.scalar.dma_start(
                                y_d[P * jt : P * (jt + 1), 512:1024],
                                ysb[:, 512:1024],
                            )
                    elif not merged:
                        if jb >= 2 and (jt + n) % 2 == 0:
                            # near/after the end of the exp stream: split
                            # drains across DVE/ACT
                            nc.scalar.copy(ysb[:, 512 * n : 512 * (n + 1)], ps)
                        else:
                            # mid-kernel drains stay off ACT (exp stream live)
                            nc.vector.tensor_copy(
                                ysb[:, 512 * n : 512 * (n + 1)], ps
                            )
                if merged:
                    # merged drains alternate DVE/ACT (exp stream is done or
                    # finishing by the time proj(2)/proj(3) drain)
                    if jl % 2 == 1:
                        nc.vector.tensor_copy(ysb, pss)
                    else:
                        nc.scalar.copy(ysb, pss)
                if jb == 3 and jl == 3:
                    # the very last tile's drains/stores went out per-half on
                    # separate engines and DGE rings above: nothing to do
                    continue
                # all projections run at the kernel tail: the sync ring is
                # free, and Pool must stay clear for the relayout SWDGE gens
                nc.sync.dma_start(y_d[P * jt : P * (jt + 1), :], ysb)

        # emission order: each block's head-pair-0 attention starts as soon
        # as its q0/k0 groups exist; qk/v chains fill attention's
        # (ACT-bound) PE slack in small pumped slices so pending score
        # matmuls are never far behind in the in-order PE queue; each
        # block's tail overlaps the next block's first scores.
        from collections import deque

        fill = deque()

        def pump(n):
            done = 0
            while fill and done < n:
                try:
                    next(fill[0])
                except StopIteration:
                    fill.popleft()
                done += 1

        def flush():
            while fill:
                pump(99)

        prev = [None]

        def start_block(tb, hp, step_cb=None, extra=None):
            g = attention_hp(tb, hp, step_cb=step_cb)
            next(g)  # emit this block's score(0)
            if prev[0] is not None:
                for _ in prev[0]:  # previous block's normalize/transpose tail
                    pass
            if extra is not None:
                extra()  # eager cross-block prep (behind score(0))
            next(g)  # this block's i-loop
            prev[0] = g

        def finish():
            for _ in prev[0]:
                pass

        # the two hp0 chains run upfront (PE is idle during the load
        # window); hp1's chains + the v chains are pumped behind hp(0,0)'s
        # scores so score(0,0) isn't queued behind 48 chain matmuls
        run_all(qk_group(0, 0))
        run_all(qk_group(2, 0))

        # v_group(g) MUST fully emit before the step that consumes v_sb[g]
        # emits its AV matmuls (the tile framework cannot depend on future
        # writes), so v chains are driven by ensure_v at their step, not by
        # the opportunistic fill queue.
        pending_v = {}

        def ensure_v(g):
            gen = pending_v.pop(g, None)
            if gen is not None:
                run_all(gen)

        for g in range(T // P):
            pending_v[g] = v_group(g, chunk=99)

        fill.append(qk_group(1, 0, chunk=2))
        fill.append(qk_group(3, 0, chunk=2))

        def cb0(i):
            # pump the (w-q1k1)-gated qk chains only from step 2 (once the
            # w-rest load has landed): pumping earlier parks them in the
            # 4-deep PE wait queue ahead of this block's own scores and
            # head-blocks the exp stream
            ensure_v(i)
            pump(6)

        start_block(0, 0, step_cb=cb0)
        flush()

        def prep_tb1():
            # t-block 0 is too short to hide t-block 1's prep in pump
            # slices: emit the fp8 chains eagerly; their matmuls wait on
            # the x-j1 load while hp(0,1)'s exps stream.
            run_all(qk_group(0, 1))
            run_all(qk_group(2, 1))

        start_block(0, 1, step_cb=lambda i: pump(5), extra=prep_tb1)
        for j in (1, 2, 3):
            flush()
            if j == 1:
                # bulk x-j2|j3 + wp loads: deferred to here so their bus
                # transfers queue behind the t-block-1 relayout DMAs
                load_tail()
            fill.append(qk_group(1, j, chunk=3))
            fill.append(qk_group(3, j, chunk=3))

            def cbj(i, j=j):
                if i >= 4 * j:
                    ensure_v(i)
                pump(5)

            start_block(j, 0, step_cb=cbj)
            flush()
            if j < 3:
                fill.append(qk_group(0, j + 1, chunk=3))
                fill.append(qk_group(2, j + 1, chunk=3))
            else:
                # during the last block DVE is otherwise idle (qkv-gen done):
                # project t-blocks 0-2 inside hp(3,1)'s exp latency so only
                # projection(3) remains on the kernel tail
                fill.append(proj_gen(0))
                fill.append(proj_gen(1))
                fill.append(proj_gen(2))
            start_block(j, 1, step_cb=lambda i, j=j: pump(3 if j == 3 else 6))
        finish()
        flush()
        projection(3)

    nc.compile()
    return nc


def _get_module():
    if "m" not in _CACHE:
        _CACHE["m"] = _build_module()
    return _CACHE["m"]


def kernel(x, w_attn, b_attn, w_proj, b_proj, **_ignored):
    import ml_dtypes
    from concourse.bass_utils import run_bass_kernel_spmd

    bfnp = np.dtype(ml_dtypes.bfloat16)
    f8np = np.dtype(ml_dtypes.float8_e4m3)
    x = np.asarray(x, dtype=np.float32)
    w_attn = np.asarray(w_attn, dtype=np.float32)
    b_attn = np.asarray(b_attn, dtype=np.float32)
    w_proj = np.asarray(w_proj, dtype=np.float32)
    b_proj = np.asarray(b_proj, dtype=np.float32)

    nc = _get_module()

    mask = np.triu(np.ones((P, P), dtype=bfnp))
    ident = np.eye(P, dtype=bfnp)
    ones = np.ones((P, HPC), dtype=bfnp)
    xT = [np.ascontiguousarray(x[b].T) for b in range(B)]
    x8 = [t.astype(f8np) for t in xT]
    xr8 = [(xT[b] - x8[b].astype(np.float32)).astype(f8np) for b in range(B)]
    # xall columns interleave per 512-t-block: [j: x8 | xr8]
    xall = []
    for b in range(B):
        blocks = []
        for j in range(TB):
            blocks.append(x8[b][:, 512 * j : 512 * (j + 1)])
            blocks.append(xr8[b][:, 512 * j : 512 * (j + 1)])
        xall.append(np.ascontiguousarray(np.concatenate(blocks, axis=1)))

    in_maps = []
    for core in range(N_CORES):
        b = core // 4
        g = core % 4
        c0 = CD * g
        wq = w_attn[:, c0 : c0 + CD]
        wk = w_attn[:, D + c0 : D + c0 + CD]
        wv = w_attn[:, 2 * D + c0 : 2 * D + c0 + CD]
        bq = b_attn[c0 : c0 + CD]
        bk = b_attn[D + c0 : D + c0 + CD]
        bvv = b_attn[2 * D + c0 : 2 * D + c0 + CD]
        # group order [q0|k0], [v], [q1|k1], pre-scaled by WS for fp8 range
        wqk0 = np.concatenate([wq[:, 0:P], wk[:, 0:P]], axis=1) * WS
        wvs = wv * WS
        wq1k1 = np.concatenate([wq[:, P:CD], wk[:, P:CD]], axis=1) * WS
        w8qk0 = wqk0.astype(f8np)
        wr8qk0 = (wqk0 - w8qk0.astype(np.float32)).astype(f8np)
        w8v = wvs.astype(f8np)
        wr8v = (wvs - w8v.astype(np.float32)).astype(f8np)
        w8q1k1 = wq1k1.astype(f8np)
        wr8q1k1 = (wq1k1 - w8q1k1.astype(np.float32)).astype(f8np)
        # wall cols: [w8 q0k0 | wr8 q0k0 | w8 v | wr8 v | w8 q1k1 | wr8 q1k1]
        wall = np.ascontiguousarray(
            np.concatenate(
                [w8qk0, wr8qk0, w8v, wr8v, w8q1k1, wr8q1k1], axis=1
            )
        )
        # packed consts: f32 [bqk 4 | bv 256] then bf16 [ident|mask|ones]
        bqk_h = np.concatenate([bq, bk]).reshape(4, P).T * WS
        bv_h = np.broadcast_to(bvv[None, :], (P, CD))
        cf32 = np.ascontiguousarray(
            np.concatenate([bqk_h, bv_h], axis=1), dtype=np.float32
        )
        cbf = np.ascontiguousarray(np.concatenate([ident, mask, ones], axis=1))
        consts = np.concatenate(
            [cf32.view(np.uint8), cbf.view(np.uint8)], axis=1
        )
        in_maps.append(
            {
                "xall": xall[b],
                "wall": wall,
                "consts": consts,
                "wp": np.ascontiguousarray(w_proj[c0 : c0 + CD, :]).astype(bfnp),
            }
        )

    res = run_bass_kernel_spmd(nc, in_maps, core_ids=list(range(N_CORES)))

    out = np.zeros((B, T, D), dtype=np.float32)
    for core in range(N_CORES):
        out[core // 4] += res.results[core]["y"].astype(np.float32)
    out += b_proj[None, None, :]
    return out
